# revision 5
# baseline (speedup 1.0000x reference)
"""Trainium2 Bass kernel: multi-head attention block (QKV proj + RoPE +
softmax attention + output proj).

Full shapes: hidden_states [4, 2048, 1024], Wq/Wk/Wv/Wo [1024, 1024],
16 heads x 64 dims. Sharding over 8 cores: data-parallel over batch (4)
x tensor-parallel over heads (2 groups of 8 heads). Each core computes a
partial output projection for its head group; the host sums the two
partials per batch.

Per-core layout strategy (everything "dim-major" = feature dim on SBUF
partitions, tokens on the free axis):
  Q^T, K^T  [512, 2048]  (8 local heads x 64 dims on partitions)
  V         [2048, 520]  token-major, 65 cols/head (col 64 = ones, so the
                          PV matmul also produces the softmax denominator)
  scores^T  = K^T-tile.T @ Q^T  -> PSUM [128 k, q], exp on ScalarE,
  attn^T    = Vpad-tile.T @ P^T -> PSUM [65, q]  (row 64 = sum of exps)
  y^T       = WoT-tile.T @ A^T  -> PSUM [128 o, t] partial output
"""

import numpy as np
import ml_dtypes

import concourse.bass as bass
import concourse.tile as tile
from concourse import bacc, mybir
from concourse.bass_utils import run_bass_kernel_spmd

BF16 = mybir.dt.bfloat16
F32 = mybir.dt.float32

B, S, H, NH, HD = 4, 2048, 1024, 16, 64
ROPE_BASE = 10000.0
N_CORES = 8
DLOC = H // 2          # 512 dims per core (8 heads)
NH_LOC = NH // 2       # 8 local heads
HT = H // 128          # 8 hidden k-tiles
DT = DLOC // 128       # 4 dim-tiles (head pairs)
TT = S // 128          # 16 token tiles
TCH = S // 512         # 4 token chunks of 512
VW = HD + 1            # 65: V columns per head incl. ones column


def _build_nc():
    nc = bacc.Bacc(None, target_bir_lowering=False)

    xt = nc.dram_tensor("xt", [H, S], BF16, kind="ExternalInput")
    wqt = nc.dram_tensor("wqt", [H, DLOC], BF16, kind="ExternalInput")
    wkt = nc.dram_tensor("wkt", [H, DLOC], BF16, kind="ExternalInput")
    wvt = nc.dram_tensor("wvt", [H, DLOC], BF16, kind="ExternalInput")
    wot = nc.dram_tensor("wot", [DLOC, H], BF16, kind="ExternalInput")
    cosd = nc.dram_tensor("cosd", [128, S], F32, kind="ExternalInput")
    sinrd = nc.dram_tensor("sinrd", [128, S], F32, kind="ExternalInput")
    yt = nc.dram_tensor("yt", [H, S], F32, kind="ExternalOutput")

    with tile.TileContext(nc) as tc:
        with tc.tile_pool(name="persist", bufs=1) as pp:
            # ---- load inputs -------------------------------------------------
            xts = []
            for i in range(HT):
                t = pp.tile([128, S], BF16, tag=f"xt{i}", name=f"xt{i}")
                nc.sync.dma_start(t[:], xt[i * 128:(i + 1) * 128, :])
                xts.append(t)
            wqts, wkts, wvts = [], [], []
            for name, dram, lst in (("wq", wqt, wqts), ("wk", wkt, wkts),
                                    ("wv", wvt, wvts)):
                for i in range(HT):
                    t = pp.tile([128, DLOC], BF16, tag=f"{name}{i}", name=f"{name}{i}")
                    nc.sync.dma_start(t[:], dram[i * 128:(i + 1) * 128, :])
                    lst.append(t)
            wots = []
            for i in range(DT):
                t = pp.tile([128, H], BF16, tag=f"wo{i}", name=f"wo{i}")
                nc.sync.dma_start(t[:], wot[i * 128:(i + 1) * 128, :])
                wots.append(t)
            cos_t = pp.tile([128, S], F32, tag="cos")
            nc.sync.dma_start(cos_t[:], cosd[:])
            sinr_t = pp.tile([128, S], F32, tag="sinr")
            nc.sync.dma_start(sinr_t[:], sinrd[:])

            qts = [pp.tile([128, S], BF16, tag=f"qt{i}", name=f"qt{i}") for i in range(DT)]
            kts = [pp.tile([128, S], BF16, tag=f"kt{i}", name=f"kt{i}") for i in range(DT)]
            vps = [pp.tile([128, NH_LOC * VW], BF16, tag=f"v{i}", name=f"v{i}")
                   for i in range(TT)]
            ats = [pp.tile([128, S], BF16, tag=f"at{i}", name=f"at{i}") for i in range(DT)]

            # ---- QKV projections + RoPE -------------------------------------
            with tc.tile_pool(name="proj_ps", bufs=4, space="PSUM") as prps, \
                 tc.tile_pool(name="rope_tmp", bufs=2) as rtp:
                for wts, outts in ((wqts, qts), (wkts, kts)):
                    for dt_i in range(DT):
                        tmp2 = rtp.tile([128, S], F32, tag="tmp2")
                        tmpc = rtp.tile([128, S], F32, tag="tmpc")
                        tmp3 = rtp.tile([128, S], F32, tag="tmp3")
                        for tch in range(TCH):
                            ps = prps.tile([128, 512], F32)
                            for ht in range(HT):
                                nc.tensor.matmul(
                                    ps[:],
                                    lhsT=wts[ht][:, dt_i * 128:(dt_i + 1) * 128],
                                    rhs=xts[ht][:, tch * 512:(tch + 1) * 512],
                                    start=(ht == 0), stop=(ht == HT - 1))
                            cs = slice(tch * 512, (tch + 1) * 512)
                            nc.vector.tensor_mul(tmp2[:, cs], ps[:], sinr_t[:, cs])
                            nc.vector.tensor_mul(tmpc[:, cs], ps[:], cos_t[:, cs])
                        # swap32: tmp3[d] = tmp2[d ^ 32]
                        for blk in (0, 64):
                            nc.sync.dma_start(tmp3[blk:blk + 32, :],
                                              tmp2[blk + 32:blk + 64, :])
                            nc.sync.dma_start(tmp3[blk + 32:blk + 64, :],
                                              tmp2[blk:blk + 32, :])
                        nc.vector.tensor_add(outts[dt_i][:], tmpc[:], tmp3[:])

                # V projection (token-major) + ones column
                for tt_i in range(TT):
                    ps = prps.tile([128, 512], F32)
                    for ht in range(HT):
                        nc.tensor.matmul(
                            ps[:],
                            lhsT=xts[ht][:, tt_i * 128:(tt_i + 1) * 128],
                            rhs=wvts[ht][:],
                            start=(ht == 0), stop=(ht == HT - 1))
                    v3 = vps[tt_i][:].rearrange("p (h d) -> p h d", d=VW)
                    p3 = ps[:].rearrange("p (h d) -> p h d", d=HD)
                    nc.vector.tensor_copy(v3[:, :, 0:HD], p3)
                    nc.vector.memset(v3[:, :, HD:VW], 1.0)

            # ---- attention ---------------------------------------------------
            with tc.tile_pool(name="qk_ps", bufs=2, space="PSUM") as qkps, \
                 tc.tile_pool(name="pv_ps", bufs=2, space="PSUM") as pvps, \
                 tc.tile_pool(name="pt", bufs=6) as ptp, \
                 tc.tile_pool(name="rc", bufs=2) as rcp:
                for h in range(NH_LOC):
                    p, r = h // 2, h % 2
                    rb = r * 64
                    for qc in range(2):
                        q0 = qc * 1024
                        pv = pvps.tile([VW, 1024], F32)
                        for kt_i in range(TT):
                            qk = qkps.tile([128, 1024], F32)
                            for j in range(2):
                                nc.tensor.matmul(
                                    qk[:, j * 512:(j + 1) * 512],
                                    lhsT=kts[p][rb:rb + 64,
                                                kt_i * 128:(kt_i + 1) * 128],
                                    rhs=qts[p][rb:rb + 64,
                                               q0 + j * 512:q0 + (j + 1) * 512],
                                    start=True, stop=True)
                            pt = ptp.tile([128, 1024], BF16)
                            nc.scalar.activation(
                                pt[:], qk[:], mybir.ActivationFunctionType.Exp,
                                scale=float(1.0 / np.sqrt(HD)))
                            for j in range(2):
                                nc.tensor.matmul(
                                    pv[:, j * 512:(j + 1) * 512],
                                    lhsT=vps[kt_i][:, h * VW:(h + 1) * VW],
                                    rhs=pt[:, j * 512:(j + 1) * 512],
                                    start=(kt_i == 0), stop=(kt_i == TT - 1))
                        sums = rcp.tile([1, 1024], F32, tag="sums")
                        nc.vector.tensor_copy(sums[:], pv[HD:VW, :])
                        recip = rcp.tile([1, 1024], F32, tag="recip")
                        nc.vector.reciprocal_approx_fast(recip[:], sums[:])
                        recip64 = rcp.tile([64, 1024], F32, tag="recip64")
                        nc.gpsimd.partition_broadcast(recip64[:], recip[:],
                                                      channels=64)
                        nc.vector.tensor_mul(
                            ats[p][rb:rb + 64, q0:q0 + 1024],
                            pv[0:HD, :], recip64[:])

            # ---- output projection ------------------------------------------
            with tc.tile_pool(name="o_ps", bufs=4, space="PSUM") as ops, \
                 tc.tile_pool(name="y", bufs=4) as yp:
                for ot in range(HT):
                    for tch in range(TCH):
                        ps = ops.tile([128, 512], F32)
                        for dt_i in range(DT):
                            nc.tensor.matmul(
                                ps[:],
                                lhsT=wots[dt_i][:, ot * 128:(ot + 1) * 128],
                                rhs=ats[dt_i][:, tch * 512:(tch + 1) * 512],
                                start=(dt_i == 0), stop=(dt_i == DT - 1))
                        ysb = yp.tile([128, 512], F32)
                        nc.scalar.copy(ysb[:], ps[:])
                        nc.sync.dma_start(
                            yt[ot * 128:(ot + 1) * 128,
                               tch * 512:(tch + 1) * 512], ysb[:])

    nc.compile()
    return nc


_NC = None


def _get_nc():
    global _NC
    if _NC is None:
        _NC = _build_nc()
    return _NC


def _host_inputs(hidden_states, Wq, Wk, Wv, Wo):
    bf = ml_dtypes.bfloat16
    inv = 1.0 / (ROPE_BASE ** (np.arange(0, HD, 2, dtype=np.float64) / HD))
    t = np.arange(S, dtype=np.float64)
    ang = np.outer(inv, t)                      # [32, S]
    cos32 = np.cos(ang).astype(np.float32)
    sin32 = np.sin(ang).astype(np.float32)
    cosd = np.tile(cos32, (4, 1))               # [128, S]
    # sinrot: +sin on lower half of each 64-block, -sin on upper half
    sinrd = np.tile(np.concatenate([sin32, -sin32], axis=0), (2, 1))
    cosd = np.ascontiguousarray(cosd, dtype=np.float32)
    sinrd = np.ascontiguousarray(sinrd, dtype=np.float32)

    WqT = np.ascontiguousarray(Wq.T).astype(bf)     # [H, H]
    WkT = np.ascontiguousarray(Wk.T).astype(bf)
    WvT = np.ascontiguousarray(Wv.T).astype(bf)
    WoT = np.ascontiguousarray(Wo.T).astype(bf)     # [H(d), H(o)]

    in_maps = []
    for c in range(N_CORES):
        b, g = c // 2, c % 2
        gs = slice(g * DLOC, (g + 1) * DLOC)
        in_maps.append({
            "xt": np.ascontiguousarray(hidden_states[b].T).astype(bf),
            "wqt": np.ascontiguousarray(WqT[:, gs]),
            "wkt": np.ascontiguousarray(WkT[:, gs]),
            "wvt": np.ascontiguousarray(WvT[:, gs]),
            "wot": np.ascontiguousarray(WoT[gs, :]),
            "cosd": cosd,
            "sinrd": sinrd,
        })
    return in_maps


def kernel(hidden_states, Wq, Wk, Wv, Wo, _trace=False, _tmpdir=None):
    nc = _get_nc()
    in_maps = _host_inputs(hidden_states, Wq, Wk, Wv, Wo)
    res = run_bass_kernel_spmd(nc, in_maps, core_ids=list(range(N_CORES)),
                               trace=_trace, tmpdir=_tmpdir)
    kernel._last_results = res
    out = np.empty((B, S, H), dtype=np.float32)
    for b in range(B):
        acc = res.results[2 * b]["yt"].astype(np.float32) \
            + res.results[2 * b + 1]["yt"].astype(np.float32)
        out[b] = acc.T
    return out


# revision 7
# speedup vs baseline: 1.3520x; 1.3520x over previous
"""Trainium2 Bass kernel: multi-head attention block (QKV proj + RoPE +
softmax attention + output proj).

Full shapes: hidden_states [4, 2048, 1024], Wq/Wk/Wv/Wo [1024, 1024],
16 heads x 64 dims. Sharding over 8 cores: data-parallel over batch (4)
x tensor-parallel over heads (2 groups of 8 heads). Each core computes a
partial output projection for its head group; the host sums the two
partials per batch.

Per-core layout strategy (everything "dim-major" = feature dim on SBUF
partitions, tokens on the free axis):
  Q^T, K^T  [512, 2048]  (8 local heads x 64 dims on partitions)
  V         [2048, 520]  token-major, 65 cols/head (col 64 = ones, so the
                          PV matmul also produces the softmax denominator)
  scores^T  = K^T-tile.T @ Q^T  -> PSUM [128 k, q], exp on ScalarE,
  attn^T    = Vpad-tile.T @ P^T -> PSUM [65, q]  (row 64 = sum of exps)
  y^T       = WoT-tile.T @ A^T  -> PSUM [128 o, t] partial output
"""

import numpy as np
import ml_dtypes

import concourse.bass as bass
import concourse.tile as tile
from concourse import bacc, mybir
from concourse.bass_utils import run_bass_kernel_spmd

BF16 = mybir.dt.bfloat16
F32 = mybir.dt.float32

B, S, H, NH, HD = 4, 2048, 1024, 16, 64
ROPE_BASE = 10000.0
N_CORES = 8
DLOC = H // 2          # 512 dims per core (8 heads)
NH_LOC = NH // 2       # 8 local heads
HT = H // 128          # 8 hidden k-tiles
DT = DLOC // 128       # 4 dim-tiles (head pairs)
TT = S // 128          # 16 token tiles
TCH = S // 512         # 4 token chunks of 512
VW = HD + 1            # 65: V columns per head incl. ones column


def _build_nc():
    nc = bacc.Bacc(None, target_bir_lowering=False)

    xt = nc.dram_tensor("xt", [H, S], BF16, kind="ExternalInput")
    wqt = nc.dram_tensor("wqt", [H, DLOC], BF16, kind="ExternalInput")
    wkt = nc.dram_tensor("wkt", [H, DLOC], BF16, kind="ExternalInput")
    wvt = nc.dram_tensor("wvt", [H, DLOC], BF16, kind="ExternalInput")
    wot = nc.dram_tensor("wot", [DLOC, H], BF16, kind="ExternalInput")
    cosd = nc.dram_tensor("cosd", [128, S], F32, kind="ExternalInput")
    sinrd = nc.dram_tensor("sinrd", [128, S], F32, kind="ExternalInput")
    yt = nc.dram_tensor("yt", [H, S], F32, kind="ExternalOutput")

    with tile.TileContext(nc) as tc:
        with tc.tile_pool(name="persist", bufs=1) as pp:
            # ---- load inputs -------------------------------------------------
            xts = []
            for i in range(HT):
                t = pp.tile([128, S], BF16, tag=f"xt{i}", name=f"xt{i}")
                nc.sync.dma_start(t[:], xt[i * 128:(i + 1) * 128, :])
                xts.append(t)
            wqts, wkts, wvts = [], [], []
            for name, dram, lst in (("wq", wqt, wqts), ("wk", wkt, wkts),
                                    ("wv", wvt, wvts)):
                for i in range(HT):
                    t = pp.tile([128, DLOC], BF16, tag=f"{name}{i}", name=f"{name}{i}")
                    nc.sync.dma_start(t[:], dram[i * 128:(i + 1) * 128, :])
                    lst.append(t)
            wots = []
            for i in range(DT):
                t = pp.tile([128, H], BF16, tag=f"wo{i}", name=f"wo{i}")
                nc.sync.dma_start(t[:], wot[i * 128:(i + 1) * 128, :])
                wots.append(t)
            cos_t = pp.tile([128, S], F32, tag="cos")
            nc.sync.dma_start(cos_t[:], cosd[:])
            sinr_t = pp.tile([128, S], F32, tag="sinr")
            nc.sync.dma_start(sinr_t[:], sinrd[:])

            qts = [pp.tile([128, S], BF16, tag=f"qt{i}", name=f"qt{i}") for i in range(DT)]
            kts = [pp.tile([128, S], BF16, tag=f"kt{i}", name=f"kt{i}") for i in range(DT)]
            vps = [pp.tile([128, NH_LOC * VW], BF16, tag=f"v{i}", name=f"v{i}")
                   for i in range(TT)]
            ats = [pp.tile([128, S], BF16, tag=f"at{i}", name=f"at{i}") for i in range(DT)]

            # ---- QKV projections + RoPE -------------------------------------
            with tc.tile_pool(name="proj_ps", bufs=4, space="PSUM") as prps, \
                 tc.tile_pool(name="rope_tmp", bufs=2) as rtp:
                for wts, outts in ((wqts, qts), (wkts, kts)):
                    for dt_i in range(DT):
                        tmp2 = rtp.tile([128, S], F32, tag="tmp2")
                        tmpc = rtp.tile([128, S], F32, tag="tmpc")
                        tmp3 = rtp.tile([128, S], F32, tag="tmp3")
                        for tch in range(TCH):
                            ps = prps.tile([128, 512], F32)
                            for ht in range(HT):
                                nc.tensor.matmul(
                                    ps[:],
                                    lhsT=wts[ht][:, dt_i * 128:(dt_i + 1) * 128],
                                    rhs=xts[ht][:, tch * 512:(tch + 1) * 512],
                                    start=(ht == 0), stop=(ht == HT - 1))
                            cs = slice(tch * 512, (tch + 1) * 512)
                            nc.vector.tensor_mul(tmp2[:, cs], ps[:], sinr_t[:, cs])
                            nc.vector.tensor_mul(tmpc[:, cs], ps[:], cos_t[:, cs])
                        # swap32: tmp3[d] = tmp2[d ^ 32]
                        for blk in (0, 64):
                            nc.sync.dma_start(tmp3[blk:blk + 32, :],
                                              tmp2[blk + 32:blk + 64, :])
                            nc.sync.dma_start(tmp3[blk + 32:blk + 64, :],
                                              tmp2[blk:blk + 32, :])
                        nc.vector.tensor_add(outts[dt_i][:], tmpc[:], tmp3[:])

                # V projection (token-major) + ones column
                for tt_i in range(TT):
                    ps = prps.tile([128, 512], F32)
                    for ht in range(HT):
                        nc.tensor.matmul(
                            ps[:],
                            lhsT=xts[ht][:, tt_i * 128:(tt_i + 1) * 128],
                            rhs=wvts[ht][:],
                            start=(ht == 0), stop=(ht == HT - 1))
                    v3 = vps[tt_i][:].rearrange("p (h d) -> p h d", d=VW)
                    p3 = ps[:].rearrange("p (h d) -> p h d", d=HD)
                    nc.vector.tensor_copy(v3[:, :, 0:HD], p3)
                    nc.vector.memset(v3[:, :, HD:VW], 1.0)

            # ---- attention ---------------------------------------------------
            with tc.tile_pool(name="qk_ps", bufs=2, space="PSUM") as qkps, \
                 tc.tile_pool(name="pv_ps", bufs=2, space="PSUM") as pvps, \
                 tc.tile_pool(name="pt", bufs=6) as ptp, \
                 tc.tile_pool(name="rc", bufs=2) as rcp:
                LAG = 3  # PV trails exp by LAG k-tiles so PE never waits on ACT
                for h in range(NH_LOC):
                    p, r = h // 2, h % 2
                    rb = r * 64
                    for qc in range(2):
                        q0 = qc * 1024
                        pv = pvps.tile([VW, 1024], F32)
                        pts = {}
                        for kt_i in range(TT + LAG):
                            if kt_i < TT:
                                qk = qkps.tile([128, 1024], F32)
                                for j in range(2):
                                    nc.tensor.matmul(
                                        qk[:, j * 512:(j + 1) * 512],
                                        lhsT=kts[p][rb:rb + 64,
                                                    kt_i * 128:(kt_i + 1) * 128],
                                        rhs=qts[p][rb:rb + 64,
                                                   q0 + j * 512:q0 + (j + 1) * 512],
                                        start=True, stop=True)
                                pt = ptp.tile([128, 1024], BF16)
                                nc.scalar.activation(
                                    pt[:], qk[:],
                                    mybir.ActivationFunctionType.Exp,
                                    scale=float(1.0 / np.sqrt(HD)))
                                pts[kt_i] = pt
                            if kt_i >= LAG:
                                kv = kt_i - LAG
                                for j in range(2):
                                    nc.tensor.matmul(
                                        pv[:, j * 512:(j + 1) * 512],
                                        lhsT=vps[kv][:, h * VW:(h + 1) * VW],
                                        rhs=pts[kv][:, j * 512:(j + 1) * 512],
                                        start=(kv == 0), stop=(kv == TT - 1))
                                del pts[kv]
                        sums = rcp.tile([1, 1024], F32, tag="sums")
                        nc.vector.tensor_copy(sums[:], pv[HD:VW, :])
                        recip = rcp.tile([1, 1024], F32, tag="recip")
                        nc.vector.reciprocal_approx_fast(recip[:], sums[:])
                        recip64 = rcp.tile([64, 1024], F32, tag="recip64")
                        nc.gpsimd.partition_broadcast(recip64[:], recip[:],
                                                      channels=64)
                        nc.vector.tensor_mul(
                            ats[p][rb:rb + 64, q0:q0 + 1024],
                            pv[0:HD, :], recip64[:])

            # ---- output projection ------------------------------------------
            with tc.tile_pool(name="o_ps", bufs=4, space="PSUM") as ops, \
                 tc.tile_pool(name="y", bufs=4) as yp:
                for ot in range(HT):
                    for tch in range(TCH):
                        ps = ops.tile([128, 512], F32)
                        for dt_i in range(DT):
                            nc.tensor.matmul(
                                ps[:],
                                lhsT=wots[dt_i][:, ot * 128:(ot + 1) * 128],
                                rhs=ats[dt_i][:, tch * 512:(tch + 1) * 512],
                                start=(dt_i == 0), stop=(dt_i == DT - 1))
                        ysb = yp.tile([128, 512], F32)
                        nc.vector.tensor_copy(ysb[:], ps[:])
                        nc.sync.dma_start(
                            yt[ot * 128:(ot + 1) * 128,
                               tch * 512:(tch + 1) * 512], ysb[:])

    nc.compile()
    return nc


_NC = None


def _get_nc():
    global _NC
    if _NC is None:
        _NC = _build_nc()
    return _NC


def _host_inputs(hidden_states, Wq, Wk, Wv, Wo):
    bf = ml_dtypes.bfloat16
    inv = 1.0 / (ROPE_BASE ** (np.arange(0, HD, 2, dtype=np.float64) / HD))
    t = np.arange(S, dtype=np.float64)
    ang = np.outer(inv, t)                      # [32, S]
    cos32 = np.cos(ang).astype(np.float32)
    sin32 = np.sin(ang).astype(np.float32)
    cosd = np.tile(cos32, (4, 1))               # [128, S]
    # sinrot: +sin on lower half of each 64-block, -sin on upper half
    sinrd = np.tile(np.concatenate([sin32, -sin32], axis=0), (2, 1))
    cosd = np.ascontiguousarray(cosd, dtype=np.float32)
    sinrd = np.ascontiguousarray(sinrd, dtype=np.float32)

    WqT = np.ascontiguousarray(Wq.T).astype(bf)     # [H, H]
    WkT = np.ascontiguousarray(Wk.T).astype(bf)
    WvT = np.ascontiguousarray(Wv.T).astype(bf)
    WoT = np.ascontiguousarray(Wo.T).astype(bf)     # [H(d), H(o)]

    in_maps = []
    for c in range(N_CORES):
        b, g = c // 2, c % 2
        gs = slice(g * DLOC, (g + 1) * DLOC)
        in_maps.append({
            "xt": np.ascontiguousarray(hidden_states[b].T).astype(bf),
            "wqt": np.ascontiguousarray(WqT[:, gs]),
            "wkt": np.ascontiguousarray(WkT[:, gs]),
            "wvt": np.ascontiguousarray(WvT[:, gs]),
            "wot": np.ascontiguousarray(WoT[gs, :]),
            "cosd": cosd,
            "sinrd": sinrd,
        })
    return in_maps


def kernel(hidden_states, Wq, Wk, Wv, Wo, _trace=False, _tmpdir=None):
    nc = _get_nc()
    in_maps = _host_inputs(hidden_states, Wq, Wk, Wv, Wo)
    res = run_bass_kernel_spmd(nc, in_maps, core_ids=list(range(N_CORES)),
                               trace=_trace, tmpdir=_tmpdir)
    kernel._last_results = res
    out = np.empty((B, S, H), dtype=np.float32)
    for b in range(B):
        acc = res.results[2 * b]["yt"].astype(np.float32) \
            + res.results[2 * b + 1]["yt"].astype(np.float32)
        out[b] = acc.T
    return out
